# revision 6
# baseline (speedup 1.0000x reference)
"""ConvNearestNeightbor Trainium2 kernel (custom fused DVE ops).

out[b, n*C+c, i, j] = max_k |x[b,c,i-r_k,j-c_k] - neighbors[n,c,k]|
over the 9 zero-padded 3x3 shifts (r_k, c_k).

Sharding: 8 cores = 4 batch-groups x 2 num-groups.
Per core: B_loc=4 batches, N_loc=16 codebook entries.
Partition layout: (nn in 0..3, c in 0..31) -> 128 partitions, with the
codebook tile index nt in 0..3 selecting n = nt*4+nn.

Compute: two custom DVE ops registered at module setup:
  ABSD_MAX_PAIR_AK: out = max(|in0-s0|, |in1-s1|)   (two planes, one op)
  ABSD_MAX_ACC_AK:  out = max(|in0-s0|, in1)        (plane + accumulator)
Per (nt, b) chain: 1 PAIR (padded window k0 x raw center k4) + 7 ACC
window ops = 8 DVE instructions for all 9 shifts, fp32 throughout.
x is DMA'd directly into a vertically-stacked padded layout
(rows 34b+1..34b+32 of a [128, 136, 34] f32 tile; shared zero pad rows
between batches) and a raw contiguous copy (for the 1D-AP center plane).
Output: plain HWDGE f32 DMA per (nt, b).
"""

import numpy as np

B, C, H, W = 16, 32, 32, 32
NUM = 32
NCORES = 8
BG, NG = 4, 2          # batch groups x num groups
B_LOC = B // BG        # 4
N_LOC = NUM // NG      # 16
NT = N_LOC // 4        # 4 codebook tiles of 4 n each
PW = 34                # padded row width
ROWS = 136             # 4 batches x 34 rows (each batch: pad,32,pad shared)

_module_cache = {}


def _register_ops():
    """Register the two fused abs-diff-max DVE ops (idempotent)."""
    import concourse.dve_ops as dve_ops
    from concourse.dve_spec import Spec, Src0, Src1, C0, C1, maxx, lower
    from concourse.dve_uop import DveOpSpec
    from concourse.dve_table_gen import dve_ver_for

    names = ("ABSD_MAX_PAIR_AK", "ABSD_MAX_ACC_AK")
    if names[0] in dve_ops._SUB_OPCODE_FOR_NAME:
        by_name = {op.name: op for op in dve_ops.OPS}
        return by_name[names[0]], by_name[names[1]]

    ver = dve_ver_for("TRN2")

    def mk(name, body, ref):
        spec = Spec(body=body, reference=ref)
        row = max(dve_ops._SUB_OPCODE_FOR_NAME.values()) + 1
        assert row < 0x20
        dve_ops._SUB_OPCODE_FOR_NAME[name] = row
        uops = lower(spec, ver=ver)
        sha = DveOpSpec(name=name, opcode=row, uops=uops, rd1_en=True).sha(ver)
        op = dve_ops.DveOp(name, spec, subdim=False, uops_sha={ver: sha})
        dve_ops.OPS.append(op)
        dve_ops.CUSTOM_DVE_SPECS[name] = spec
        return op

    pair = mk(
        names[0],
        maxx(maxx(Src0 - C0, C0 - Src0), maxx(Src1 - C1, C1 - Src1)),
        lambda in0, in1, s0, s1, imm2: np.maximum(
            np.abs(in0.astype(np.float32) - s0),
            np.abs(in1.astype(np.float32).reshape(in0.shape) - s1),
        ),
    )
    acc = mk(
        names[1],
        maxx(maxx(Src0 - C0, C0 - Src0), Src1),
        lambda in0, in1, s0, s1, imm2: np.maximum(
            np.abs(in0.astype(np.float32) - s0),
            in1.astype(np.float32).reshape(in0.shape),
        ),
    )
    return pair, acc


def _build_module():
    import concourse.bacc as bacc
    import concourse.mybir as mybir
    import concourse.tile as tile

    PAIR, ACC = _register_ops()

    dt = mybir.dt

    nc = bacc.Bacc("TRN2", debug=False)
    x = nc.dram_tensor("x", [B_LOC, C, H, W], dt.float32, kind="ExternalInput")
    nb = nc.dram_tensor("neighbors", [N_LOC, C, 9], dt.float32, kind="ExternalInput")
    out = nc.dram_tensor(
        "out", [B_LOC, N_LOC * C, H, W], dt.float32, kind="ExternalOutput"
    )

    # window row/col offsets within a batch's 34x34 padded block:
    # k = (row+1)*3 + (col+1), window starts at (1-row, 1-col)
    offs = []
    for row in (-1, 0, 1):
        for col in (-1, 0, 1):
            offs.append((1 - row, 1 - col))

    with tile.TileContext(nc) as tc:
        with (
            tc.tile_pool(name="const", bufs=1) as cpool,
            tc.tile_pool(name="pp", bufs=4) as ppool,
        ):
            # neighbors scalars first (tiny DMA; gates first PAIR)
            nbt = cpool.tile([128, NT * 9], dt.float32, tag="nbt")
            nb_src = nb.ap().rearrange("(t nn) c k -> (nn c) t k", nn=4)
            nbt_v = nbt[:].rearrange("p (t k) -> p t k", t=NT)
            nc.sync.dma_start(nbt_v, nb_src)

            def nbcol(nt, k):
                return nbt[:, nt * 9 + k : nt * 9 + k + 1]

            # raw contiguous x copy: [p, b, (h w)] f32
            xraw = cpool.tile([128, B_LOC * H * W], dt.float32, tag="xraw")
            xraw_v = xraw[:].rearrange("p (b s) -> p b s", b=B_LOC)
            x_src = x.ap().rearrange("b c h w -> c b (h w)")
            for b in range(B_LOC):
                for nn in range(4):
                    eng = nc.sync if nn % 2 == 0 else nc.scalar
                    eng.dma_start(
                        xraw_v[nn * 32 : (nn + 1) * 32, b], x_src[:, b]
                    )

            # padded tile: [p, 136, 34] f32, batch b interior at rows
            # 34b+1..34b+32, cols 1..32; border rows/cols zero
            xpad = cpool.tile([128, ROWS * PW], dt.float32, tag="xpad")
            xp = xpad[:].rearrange("p (r w) -> p r w", r=ROWS)
            # border memsets (rows 0,33,34,67,68,101,102,135 + cols 0,33)
            nc.gpsimd.memset(xp[:, 0:1, :], 0.0)
            for b in range(1, B_LOC):
                nc.gpsimd.memset(xp[:, 34 * b - 1 : 34 * b + 1, :], 0.0)
            nc.gpsimd.memset(xp[:, ROWS - 1 : ROWS, :], 0.0)
            nc.gpsimd.memset(xp[:, :, 0:1], 0.0)
            nc.gpsimd.memset(xp[:, :, 33:34], 0.0)

            # interior loads, per (b, nn)
            x_src4 = x.ap()  # [b, c, h, w]
            for b in range(B_LOC):
                for nn in range(4):
                    eng = nc.sync if nn % 2 == 1 else nc.scalar
                    eng.dma_start(
                        xp[nn * 32 : (nn + 1) * 32, 34 * b + 1 : 34 * b + 33, 1:33],
                        x_src4[b],
                    )

            def win(b, k):
                a, c = offs[k]
                return xp[:, 34 * b + a : 34 * b + a + 32, c : c + 32]

            # accumulators, one per nt: [p, b, (h w)] f32
            accs = [
                cpool.tile(
                    [128, B_LOC * H * W], dt.float32, tag=f"acc{nt}",
                    name=f"acc{nt}",
                )
                for nt in range(NT)
            ]
            out_v = out.ap().rearrange("b (t p) h w -> t p b (h w)", t=NT)

            K_REST = (1, 2, 3, 5, 6, 7, 8)
            dma_engs = [nc.sync, nc.scalar, nc.gpsimd, nc.sync]

            for nt in range(NT):
                acc_v = accs[nt][:].rearrange("p (b s) -> p b s", b=B_LOC)
                for b in range(B_LOC):
                    p = ppool.tile([128, H * W], dt.float32, tag="p")
                    nc.vector._custom_dve(
                        PAIR, out=p[:], in0=win(b, 0), in1=xraw_v[:, b],
                        s0=nbcol(nt, 0), s1=nbcol(nt, 4),
                    )
                    dst = acc_v[:, b]
                    src1 = p[:]
                    for k in K_REST:
                        nc.vector._custom_dve(
                            ACC, out=dst, in0=win(b, k), in1=src1,
                            s0=nbcol(nt, k),
                        )
                        src1 = dst
                    dma_engs[b].dma_start(out_v[nt][:, b], acc_v[:, b])

    nc.compile()
    return nc


def _get_module():
    if "nc" not in _module_cache:
        _module_cache["nc"] = _build_module()
    return _module_cache["nc"]


def _run(x, neighbors, trace=False):
    from concourse import bass_utils

    x = np.ascontiguousarray(x, dtype=np.float32)
    neighbors = np.ascontiguousarray(neighbors, dtype=np.float32)
    in_maps = []
    for core in range(NCORES):
        bg, ng = divmod(core, NG)
        in_maps.append(
            {
                "x": x[bg * B_LOC : (bg + 1) * B_LOC],
                "neighbors": neighbors[ng * N_LOC : (ng + 1) * N_LOC],
            }
        )
    res = bass_utils.run_bass_kernel_spmd(
        _get_module(), in_maps, core_ids=list(range(NCORES)), trace=trace
    )
    out = np.empty((B, NUM * C, H, W), dtype=np.float32)
    for core in range(NCORES):
        bg, ng = divmod(core, NG)
        out[bg * B_LOC : (bg + 1) * B_LOC, ng * N_LOC * C : (ng + 1) * N_LOC * C] = (
            res.results[core]["out"]
        )
    return out, res


def kernel(x, neighbors):
    out, _ = _run(x, neighbors, trace=False)
    return out


# revision 8
# speedup vs baseline: 1.2538x; 1.2538x over previous
"""ConvNearestNeightbor Trainium2 kernel (hybrid ACT + custom fused DVE ops).

out[b, n*C+c, i, j] = max_k |x[b,c,i-r_k,j-c_k] - neighbors[n,c,k]|
over the 9 zero-padded 3x3 shifts (r_k, c_k).

Sharding: 8 cores = 4 batch-groups x 2 num-groups.
Per core: B_loc=4 batches, N_loc=16 codebook entries.
Partition layout: (nn in 0..3, c in 0..31) -> 128 partitions, with the
codebook tile index nt in 0..3 selecting n = nt*4+nn.

Engine split per (nt) chain (measured rates):
 - DVE custom ops (1 elem/cycle, fused produce+abs+fold):
     ABSD_MAX_PAIR_AK: out = max(|in0-s0|, |in1-s1|)  -> planes k0+k4
     ABSD_MAX_ACC_AK:  out = max(|in0-s0|, in1)       -> plane k2
 - ACT produces the other 6 planes as |x-n| (Abs+bias, f32 window in,
   bf16 out), which DVE folds with bf16 tensor_tensor max (2 elem/cyc).
x lives in a vertically-stacked padded f32 tile ([128, 136, 34]; batch b
interior at rows 34b+1..34b+32, shared zero pad rows between batches)
plus a raw contiguous f32 copy (1D AP for the PAIR's second plane).
Output: bf16 accumulator -> SWDGE cast DMA -> f32.
"""

import numpy as np

B, C, H, W = 16, 32, 32, 32
NUM = 32
NCORES = 8
BG, NG = 4, 2          # batch groups x num groups
B_LOC = B // BG        # 4
N_LOC = NUM // NG      # 16
NT = N_LOC // 4        # 4 codebook tiles of 4 n each
PW = 34                # padded row width
ROWS = 136             # 4 batches x 34 rows (pad rows shared)

K_DVE = (0, 4, 2)      # PAIR(k0, k4) then ACC(k2)
K_ACT = (1, 3, 5, 6, 7, 8)

_module_cache = {}


def _register_ops():
    """Register the two fused abs-diff-max DVE ops (idempotent)."""
    import concourse.dve_ops as dve_ops
    from concourse.dve_spec import Spec, Src0, Src1, C0, C1, maxx, lower
    from concourse.dve_uop import DveOpSpec
    from concourse.dve_table_gen import dve_ver_for

    names = ("ABSD_MAX_PAIR_AK", "ABSD_MAX_ACC_AK")
    if names[0] in dve_ops._SUB_OPCODE_FOR_NAME:
        by_name = {op.name: op for op in dve_ops.OPS}
        return by_name[names[0]], by_name[names[1]]

    ver = dve_ver_for("TRN2")

    def mk(name, body, ref):
        spec = Spec(body=body, reference=ref)
        row = max(dve_ops._SUB_OPCODE_FOR_NAME.values()) + 1
        assert row < 0x20
        dve_ops._SUB_OPCODE_FOR_NAME[name] = row
        uops = lower(spec, ver=ver)
        sha = DveOpSpec(name=name, opcode=row, uops=uops, rd1_en=True).sha(ver)
        op = dve_ops.DveOp(name, spec, subdim=False, uops_sha={ver: sha})
        dve_ops.OPS.append(op)
        dve_ops.CUSTOM_DVE_SPECS[name] = spec
        return op

    pair = mk(
        names[0],
        maxx(maxx(Src0 - C0, C0 - Src0), maxx(Src1 - C1, C1 - Src1)),
        lambda in0, in1, s0, s1, imm2: np.maximum(
            np.abs(in0.astype(np.float32) - s0),
            np.abs(in1.astype(np.float32).reshape(in0.shape) - s1),
        ),
    )
    acc = mk(
        names[1],
        maxx(maxx(Src0 - C0, C0 - Src0), Src1),
        lambda in0, in1, s0, s1, imm2: np.maximum(
            np.abs(in0.astype(np.float32) - s0),
            in1.astype(np.float32).reshape(in0.shape),
        ),
    )
    return pair, acc


def _build_module():
    import concourse.bacc as bacc
    import concourse.mybir as mybir
    import concourse.tile as tile

    PAIR, ACC = _register_ops()

    dt = mybir.dt
    Alu = mybir.AluOpType
    AF = mybir.ActivationFunctionType

    nc = bacc.Bacc("TRN2", debug=False)
    x = nc.dram_tensor("x", [B_LOC, C, H, W], dt.float32, kind="ExternalInput")
    nb = nc.dram_tensor("neighbors", [N_LOC, C, 9], dt.float32, kind="ExternalInput")
    out = nc.dram_tensor(
        "out", [B_LOC, N_LOC * C, H, W], dt.float32, kind="ExternalOutput"
    )

    # k = (row+1)*3 + (col+1), window starts at (1-row, 1-col)
    offs = []
    for row in (-1, 0, 1):
        for col in (-1, 0, 1):
            offs.append((1 - row, 1 - col))

    with tile.TileContext(nc) as tc:
        with (
            tc.tile_pool(name="const", bufs=1) as cpool,
            tc.tile_pool(name="pp", bufs=3) as ppool,
            tc.tile_pool(name="dp", bufs=2) as dpool,
        ):
            # neighbors scalars first (tiny DMA; gates first ops)
            nbt = cpool.tile([128, NT * 9], dt.float32, tag="nbt")
            nb_src = nb.ap().rearrange("(t nn) c k -> (nn c) t k", nn=4)
            nbt_v = nbt[:].rearrange("p (t k) -> p t k", t=NT)
            nc.sync.dma_start(nbt_v, nb_src)
            nbneg = cpool.tile([128, NT * 9], dt.float32, tag="nbneg")
            nc.scalar.mul(nbneg[:], nbt[:], -1.0)

            def nbcol(nt, k):
                return nbt[:, nt * 9 + k : nt * 9 + k + 1]

            def nbnegcol(nt, k):
                return nbneg[:, nt * 9 + k : nt * 9 + k + 1]

            # padded tile [p, 136, 34] f32; border memsets, then interiors
            xpad = cpool.tile([128, ROWS * PW], dt.float32, tag="xpad")
            xp = xpad[:].rearrange("p (r w) -> p r w", r=ROWS)
            nc.gpsimd.memset(xp[:, 0:1, :], 0.0)
            for b in range(1, B_LOC):
                nc.gpsimd.memset(xp[:, 34 * b - 1 : 34 * b + 1, :], 0.0)
            nc.gpsimd.memset(xp[:, ROWS - 1 : ROWS, :], 0.0)
            nc.gpsimd.memset(xp[:, :, 0:1], 0.0)
            nc.gpsimd.memset(xp[:, :, 33:34], 0.0)

            # raw contiguous x copy: [p, b, (h w)] f32
            xraw = cpool.tile([128, B_LOC * H * W], dt.float32, tag="xraw")
            xraw_v = xraw[:].rearrange("p (b s) -> p b s", b=B_LOC)
            x_src = x.ap().rearrange("b c h w -> c b (h w)")

            # b-major loads: for each b, interior (4 nn) + raw (4 nn), so
            # batch 0 is complete as early as possible.
            engs = [nc.sync, nc.scalar]
            for b in range(B_LOC):
                for nn in range(4):
                    engs[nn % 2].dma_start(
                        xp[nn * 32 : (nn + 1) * 32, 34 * b + 1 : 34 * b + 33, 1:33],
                        x.ap()[b],
                    )
                for nn in range(4):
                    engs[(nn + 1) % 2].dma_start(
                        xraw_v[nn * 32 : (nn + 1) * 32, b], x_src[:, b]
                    )

            xp4 = xpad[:].rearrange("p (b r w) -> p b r w", b=B_LOC, r=34)

            def win(b, k):
                a, c = offs[k]
                return xp[:, 34 * b + a : 34 * b + a + 32, c : c + 32]

            def win2(bb, k):
                # two-batch window view [p, 2, 32, 32] for ACT (bb = 0 or 1)
                a, c = offs[k]
                return xp4[:, 2 * bb : 2 * bb + 2, a : a + 32, c : c + 32]

            accs = [
                cpool.tile(
                    [128, B_LOC * H * W], dt.bfloat16, tag=f"acc{nt}",
                    name=f"acc{nt}",
                )
                for nt in range(NT)
            ]
            out_v = out.ap().rearrange("b (t p) h w -> t p b (h w)", t=NT)

            for nt in range(NT):
                acc_v = accs[nt][:].rearrange("p (b s) -> p b s", b=B_LOC)
                # ACT planes: 6 x |x - n| at 2048 (two batches per op)
                ds = []
                for ki, k in enumerate(K_ACT):
                    d = dpool.tile(
                        [128, B_LOC * H * W], dt.bfloat16, tag=f"d{ki}",
                        name=f"d{ki}_{nt}",
                    )
                    d_v = d[:].rearrange("p (b s) -> p b s", b=B_LOC)
                    for bb in range(2):
                        nc.scalar.activation(
                            d_v[:, 2 * bb : 2 * bb + 2].rearrange(
                                "p b (h w) -> p b h w", h=H
                            ),
                            win2(bb, k),
                            AF.Abs, bias=nbnegcol(nt, k), scale=1.0,
                        )
                    ds.append(d)
                # DVE custom chains per batch: PAIR(k0, k4) -> ACC(k2)
                for b in range(B_LOC):
                    p = ppool.tile([128, H * W], dt.bfloat16, tag="p")
                    nc.vector._custom_dve(
                        PAIR, out=p[:], in0=win(b, 0), in1=xraw_v[:, b],
                        s0=nbcol(nt, 0), s1=nbcol(nt, 4),
                    )
                    nc.vector._custom_dve(
                        ACC, out=acc_v[:, b], in0=win(b, 2), in1=p[:],
                        s0=nbcol(nt, 2),
                    )
                # bf16 TT folds at 4096 (in-place)
                d0, d1, d2, d3, d4, d5 = ds
                nc.vector.tensor_tensor(d0[:], d0[:], d1[:], Alu.max)
                nc.vector.tensor_tensor(d2[:], d2[:], d3[:], Alu.max)
                nc.vector.tensor_tensor(d4[:], d4[:], d5[:], Alu.max)
                nc.vector.tensor_tensor(d0[:], d0[:], d2[:], Alu.max)
                nc.vector.tensor_tensor(accs[nt][:], accs[nt][:], d4[:], Alu.max)
                nc.vector.tensor_tensor(accs[nt][:], accs[nt][:], d0[:], Alu.max)
                # stream out (SWDGE cast bf16 -> f32)
                acc_s = accs[nt][:].rearrange("p (b s) -> p b s", b=B_LOC)
                nc.gpsimd.dma_start(out_v[nt], acc_s)

    nc.compile()
    return nc


def _get_module():
    if "nc" not in _module_cache:
        _module_cache["nc"] = _build_module()
    return _module_cache["nc"]


def _run(x, neighbors, trace=False):
    from concourse import bass_utils

    x = np.ascontiguousarray(x, dtype=np.float32)
    neighbors = np.ascontiguousarray(neighbors, dtype=np.float32)
    in_maps = []
    for core in range(NCORES):
        bg, ng = divmod(core, NG)
        in_maps.append(
            {
                "x": x[bg * B_LOC : (bg + 1) * B_LOC],
                "neighbors": neighbors[ng * N_LOC : (ng + 1) * N_LOC],
            }
        )
    res = bass_utils.run_bass_kernel_spmd(
        _get_module(), in_maps, core_ids=list(range(NCORES)), trace=trace
    )
    out = np.empty((B, NUM * C, H, W), dtype=np.float32)
    for core in range(NCORES):
        bg, ng = divmod(core, NG)
        out[bg * B_LOC : (bg + 1) * B_LOC, ng * N_LOC * C : (ng + 1) * N_LOC * C] = (
            res.results[core]["out"]
        )
    return out, res


def kernel(x, neighbors):
    out, _ = _run(x, neighbors, trace=False)
    return out


# revision 10
# speedup vs baseline: 1.2561x; 1.0019x over previous
"""ConvNearestNeightbor Trainium2 kernel (hybrid ACT + custom fused DVE ops).

out[b, n*C+c, i, j] = max_k |x[b,c,i-r_k,j-c_k] - neighbors[n,c,k]|
over the 9 zero-padded 3x3 shifts (r_k, c_k).

Sharding: 8 cores = 4 batch-groups x 2 num-groups.
Per core: B_loc=4 batches, N_loc=16 codebook entries.
Partition layout: (nn in 0..3, c in 0..31) -> 128 partitions; codebook
tile nt in 0..3 selects n = nt*4+nn.

x layout: NO column padding -- a [128, 135*32] f32 tile, batch b interior
contiguous at rows 2+33b .. 33+33b (32x32), zero pad rows between
batches (vertical zero-padding is exact).  Every 3x3-shift "window" is
then a 1D contiguous crop at offset (2+33b-r)*32 - c, so input DMA is
4KB-run cheap and custom-DVE ops take windows as both operands.
Horizontal shifts wrap one column per row into the neighbouring row;
that single known column per shifted plane is overwritten with the
correct zero-pad value |n_k| (tiny strided tensor_scalar) before
folding.

Engine split per nt (measured rates: ACT 1.2G elem/s; DVE custom 1 elem/
cyc fused produce+abs+fold, TT bf16 2 elem/cyc):
 - ACT: planes {0,2,3,5,6,8} as |x-n| (Abs+bias, f32 in, bf16 out)
 - DVE: PAIR(k1,k7) per batch + ACC(k4) at 4096, column fixups, and
   6 bf16 tensor_tensor max folds at 4096.
Output: bf16 accumulator -> SWDGE cast DMA -> f32.
"""

import numpy as np

B, C, H, W = 16, 32, 32, 32
NUM = 32
NCORES = 8
BG, NG = 4, 2          # batch groups x num groups
B_LOC = B // BG        # 4
N_LOC = NUM // NG      # 16
NT = N_LOC // 4        # 4 codebook tiles of 4 n each
ROWS = 136             # guard + 4x(pad+32) + bottom pads/guards
FREE = ROWS * 32

K_ACT = (0, 2, 3, 5, 6, 8)

_module_cache = {}


def _register_ops():
    """Register the two fused abs-diff-max DVE ops (idempotent)."""
    import concourse.dve_ops as dve_ops
    from concourse.dve_spec import Spec, Src0, Src1, C0, C1, maxx, lower
    from concourse.dve_uop import DveOpSpec
    from concourse.dve_table_gen import dve_ver_for

    names = ("ABSD_MAX_PAIR_AK", "ABSD_MAX_ACC_AK")
    if names[0] in dve_ops._SUB_OPCODE_FOR_NAME:
        by_name = {op.name: op for op in dve_ops.OPS}
        return by_name[names[0]], by_name[names[1]]

    ver = dve_ver_for("TRN2")

    def mk(name, body, ref):
        spec = Spec(body=body, reference=ref)
        row = max(dve_ops._SUB_OPCODE_FOR_NAME.values()) + 1
        assert row < 0x20
        dve_ops._SUB_OPCODE_FOR_NAME[name] = row
        uops = lower(spec, ver=ver)
        sha = DveOpSpec(name=name, opcode=row, uops=uops, rd1_en=True).sha(ver)
        op = dve_ops.DveOp(name, spec, subdim=False, uops_sha={ver: sha})
        dve_ops.OPS.append(op)
        dve_ops.CUSTOM_DVE_SPECS[name] = spec
        return op

    pair = mk(
        names[0],
        maxx(maxx(Src0 - C0, C0 - Src0), maxx(Src1 - C1, C1 - Src1)),
        lambda in0, in1, s0, s1, imm2: np.maximum(
            np.abs(in0.astype(np.float32) - s0),
            np.abs(in1.astype(np.float32).reshape(in0.shape) - s1),
        ),
    )
    acc = mk(
        names[1],
        maxx(maxx(Src0 - C0, C0 - Src0), Src1),
        lambda in0, in1, s0, s1, imm2: np.maximum(
            np.abs(in0.astype(np.float32) - s0),
            in1.astype(np.float32).reshape(in0.shape),
        ),
    )
    return pair, acc


def _build_module():
    import concourse.bacc as bacc
    import concourse.mybir as mybir
    import concourse.tile as tile

    PAIR, ACC = _register_ops()

    dt = mybir.dt
    Alu = mybir.AluOpType
    AF = mybir.ActivationFunctionType

    nc = bacc.Bacc("TRN2", debug=False)
    x = nc.dram_tensor("x", [B_LOC, C, H, W], dt.float32, kind="ExternalInput")
    nb = nc.dram_tensor("neighbors", [N_LOC, C, 9], dt.float32, kind="ExternalInput")
    out = nc.dram_tensor(
        "out", [B_LOC, N_LOC * C, H, W], dt.float32, kind="ExternalOutput"
    )

    # shift k = (r+1)*3 + (c+1) with r, c in {-1, 0, 1}
    RC = [(r, c) for r in (-1, 0, 1) for c in (-1, 0, 1)]

    def wstart(b, k):
        r, c = RC[k]
        return (2 + 33 * b - r) * 32 - c

    # bad (wrapped) column of a c-shifted plane: c=+1 -> col 0, c=-1 -> col 31
    def badcol(k):
        c = RC[k][1]
        return None if c == 0 else (0 if c == 1 else 31)

    with tile.TileContext(nc) as tc:
        with (
            tc.tile_pool(name="const", bufs=1) as cpool,
            tc.tile_pool(name="pp", bufs=3) as ppool,
            tc.tile_pool(name="dp", bufs=2) as dpool,
        ):
            # neighbors scalars first (tiny DMA; gates first ops)
            nbt = cpool.tile([128, NT * 9], dt.float32, tag="nbt")
            nb_src = nb.ap().rearrange("(t nn) c k -> (nn c) t k", nn=4)
            nc.sync.dma_start(nbt[:].rearrange("p (t k) -> p t k", t=NT), nb_src)
            nbneg = cpool.tile([128, NT * 9], dt.float32, tag="nbneg")
            nc.scalar.mul(nbneg[:], nbt[:], -1.0)
            nbabs = cpool.tile([128, NT * 9], dt.float32, tag="nbabs")
            nc.scalar.activation(nbabs[:], nbt[:], AF.Abs, scale=1.0)

            def nbcol(nt, k):
                return nbt[:, nt * 9 + k : nt * 9 + k + 1]

            def nbnegcol(nt, k):
                return nbneg[:, nt * 9 + k : nt * 9 + k + 1]

            def nbabscol(nt, k):
                return nbabs[:, nt * 9 + k : nt * 9 + k + 1]

            # padded x tile and pad-row memsets
            xpad = cpool.tile([128, FREE], dt.float32, tag="xpad")
            xr = xpad[:].rearrange("p (r w) -> p r w", r=ROWS)
            nc.gpsimd.memset(xr[:, 0:2, :], 0.0)        # guard + top pad
            for b in range(1, B_LOC):
                nc.gpsimd.memset(xr[:, 33 * b + 1 : 33 * b + 2, :], 0.0)
            nc.gpsimd.memset(xr[:, ROWS - 3 : ROWS, :], 0.0)

            # interior loads (contiguous 1024 f32 per (b, c)): b-major
            engs = [nc.sync, nc.scalar]
            for b in range(B_LOC):
                for nn in range(4):
                    engs[nn % 2].dma_start(
                        xr[nn * 32 : (nn + 1) * 32, 2 + 33 * b : 34 + 33 * b, :],
                        x.ap()[b],
                    )

            def win(b, k):
                s = wstart(b, k)
                return xpad[:, s : s + 1024]

            def win4(k):
                # all-batch window [p, 4, 1024] (stride 33*32)
                s = wstart(0, k)
                return xpad[:, s : s + 4 * 1056].rearrange(
                    "p (b u) -> p b u", u=1056
                )[:, :, :1024]

            accs = [
                cpool.tile(
                    [128, B_LOC * H * W], dt.bfloat16, tag=f"acc{nt}",
                    name=f"acc{nt}",
                )
                for nt in range(NT)
            ]
            out_v = out.ap().rearrange("b (t p) h w -> t p b (h w)", t=NT)

            for nt in range(NT):
                # ACT planes: |x - n| at 4096 (bf16 out)
                ds = []
                for ki, k in enumerate(K_ACT):
                    d = dpool.tile(
                        [128, B_LOC * H * W], dt.bfloat16, tag=f"d{ki}",
                        name=f"d{ki}_{nt}",
                    )
                    nc.scalar.activation(
                        d[:].rearrange("p (b u) -> p b u", b=B_LOC),
                        win4(k), AF.Abs, bias=nbnegcol(nt, k), scale=1.0,
                    )
                    ds.append(d)

                # DVE chains: PAIR(k1, k7) per batch into pslab, ACC(k4)
                pslab = ppool.tile(
                    [128, B_LOC * H * W], dt.bfloat16, tag="pslab"
                )
                ps_v = pslab[:].rearrange("p (b u) -> p b u", b=B_LOC)
                for b in range(B_LOC):
                    nc.vector._custom_dve(
                        PAIR, out=ps_v[:, b], in0=win(b, 1), in1=win(b, 7),
                        s0=nbcol(nt, 1), s1=nbcol(nt, 7),
                    )
                nc.vector._custom_dve(
                    ACC, out=accs[nt][:], in0=win4(4), in1=pslab[:],
                    s0=nbcol(nt, 4),
                )

                # column fixups on ACT planes (bad wrapped column <- |n_k|)
                for ki, k in enumerate(K_ACT):
                    col = badcol(k)
                    dv = ds[ki][:].rearrange(
                        "p (b h w) -> p b h w", b=B_LOC, h=H
                    )[:, :, :, col : col + 1]
                    nc.vector.tensor_scalar(
                        dv, dv, 0.0, nbabscol(nt, k), Alu.mult, Alu.add
                    )

                # bf16 TT max folds at 4096 (eager pairs, then into acc)
                d0, d1, d2, d3, d4, d5 = ds
                nc.vector.tensor_tensor(d0[:], d0[:], d1[:], Alu.max)
                nc.vector.tensor_tensor(d2[:], d2[:], d3[:], Alu.max)
                nc.vector.tensor_tensor(d4[:], d4[:], d5[:], Alu.max)
                nc.vector.tensor_tensor(accs[nt][:], accs[nt][:], d0[:], Alu.max)
                nc.vector.tensor_tensor(accs[nt][:], accs[nt][:], d2[:], Alu.max)
                nc.vector.tensor_tensor(accs[nt][:], accs[nt][:], d4[:], Alu.max)

                # stream out (SWDGE cast bf16 -> f32)
                acc_s = accs[nt][:].rearrange("p (b s) -> p b s", b=B_LOC)
                nc.gpsimd.dma_start(out_v[nt], acc_s)

    nc.compile()
    return nc


def _get_module():
    if "nc" not in _module_cache:
        _module_cache["nc"] = _build_module()
    return _module_cache["nc"]


def _run(x, neighbors, trace=False):
    from concourse import bass_utils

    x = np.ascontiguousarray(x, dtype=np.float32)
    neighbors = np.ascontiguousarray(neighbors, dtype=np.float32)
    in_maps = []
    for core in range(NCORES):
        bg, ng = divmod(core, NG)
        in_maps.append(
            {
                "x": x[bg * B_LOC : (bg + 1) * B_LOC],
                "neighbors": neighbors[ng * N_LOC : (ng + 1) * N_LOC],
            }
        )
    res = bass_utils.run_bass_kernel_spmd(
        _get_module(), in_maps, core_ids=list(range(NCORES)), trace=trace
    )
    out = np.empty((B, NUM * C, H, W), dtype=np.float32)
    for core in range(NCORES):
        bg, ng = divmod(core, NG)
        out[bg * B_LOC : (bg + 1) * B_LOC, ng * N_LOC * C : (ng + 1) * N_LOC * C] = (
            res.results[core]["out"]
        )
    return out, res


def kernel(x, neighbors):
    out, _ = _run(x, neighbors, trace=False)
    return out


# revision 17
# speedup vs baseline: 1.5153x; 1.2063x over previous
"""ConvNearestNeightbor Trainium2 kernel (hybrid ACT + custom fused DVE ops).

out[b, n*C+c, i, j] = max_k |x[b,c,i-r_k,j-c_k] - neighbors[n,c,k]|
over the 9 zero-padded 3x3 shifts (r_k, c_k).

Sharding: 8 cores = 4 batch-groups x 2 num-groups.
Per core: B_loc=4 batches, N_loc=16 codebook entries.
Partition layout: (nn in 0..3, c in 0..31) -> 128 partitions; codebook
tile nt in 0..3 selects n = nt*4+nn.

x layout: NO column padding -- a [128, 135*32] f32 tile, batch b interior
contiguous at rows 2+33b .. 33+33b (32x32), zero pad rows between
batches (vertical zero-padding is exact).  Every 3x3-shift "window" is
then a 1D contiguous crop at offset (2+33b-r)*32 - c, so input DMA is
4KB-run cheap and custom-DVE ops take windows as both operands.
Horizontal shifts wrap one column per row into the neighbouring row;
that single known column per shifted plane is overwritten with the
correct zero-pad value |n_k| (tiny strided tensor_scalar) before
folding.

Engine split per nt (measured rates: ACT 1.2G elem/s; DVE custom 1 elem/
cyc fused produce+abs+fold, TT bf16 2 elem/cyc):
 - ACT: planes {0,2,3,5,6,8} as |x-n| (Abs+bias, f32 in, bf16 out)
 - DVE: PAIR(k1,k7) per batch + ACC(k4) at 4096, column fixups, and
   6 bf16 tensor_tensor max folds at 4096.
Output: bf16 accumulator -> SWDGE cast DMA -> f32.
"""

import numpy as np

B, C, H, W = 16, 32, 32, 32
NUM = 32
NCORES = 8
BG, NG = 4, 2          # batch groups x num groups
B_LOC = B // BG        # 4
N_LOC = NUM // NG      # 16
NT = N_LOC // 4        # 4 codebook tiles of 4 n each
ROWS = 136             # guard + 4x(pad+32) + bottom pads/guards
FREE = ROWS * 32

K_ACT = (0, 2, 3, 5, 6, 8)

_module_cache = {}


def _register_ops():
    """Register the two fused abs-diff-max DVE ops (idempotent)."""
    import concourse.dve_ops as dve_ops
    from concourse.dve_spec import Spec, Src0, Src1, C0, C1, maxx, lower
    from concourse.dve_uop import DveOpSpec
    from concourse.dve_table_gen import dve_ver_for

    names = ("ABSD_MAX_PAIR_AK", "ABSD_MAX_ACC_AK")
    if names[0] in dve_ops._SUB_OPCODE_FOR_NAME:
        by_name = {op.name: op for op in dve_ops.OPS}
        return by_name[names[0]], by_name[names[1]]

    ver = dve_ver_for("TRN2")

    def mk(name, body, ref):
        spec = Spec(body=body, reference=ref)
        row = max(dve_ops._SUB_OPCODE_FOR_NAME.values()) + 1
        assert row < 0x20
        dve_ops._SUB_OPCODE_FOR_NAME[name] = row
        uops = lower(spec, ver=ver)
        sha = DveOpSpec(name=name, opcode=row, uops=uops, rd1_en=True).sha(ver)
        op = dve_ops.DveOp(name, spec, subdim=False, uops_sha={ver: sha})
        dve_ops.OPS.append(op)
        dve_ops.CUSTOM_DVE_SPECS[name] = spec
        return op

    pair = mk(
        names[0],
        maxx(maxx(Src0 - C0, C0 - Src0), maxx(Src1 - C1, C1 - Src1)),
        lambda in0, in1, s0, s1, imm2: np.maximum(
            np.abs(in0.astype(np.float32) - s0),
            np.abs(in1.astype(np.float32).reshape(in0.shape) - s1),
        ),
    )
    acc = mk(
        names[1],
        maxx(maxx(Src0 - C0, C0 - Src0), Src1),
        lambda in0, in1, s0, s1, imm2: np.maximum(
            np.abs(in0.astype(np.float32) - s0),
            in1.astype(np.float32).reshape(in0.shape),
        ),
    )
    return pair, acc


def _build_module():
    import concourse.bacc as bacc
    import concourse.mybir as mybir
    import concourse.tile as tile

    PAIR, ACC = _register_ops()

    dt = mybir.dt
    Alu = mybir.AluOpType
    AF = mybir.ActivationFunctionType

    nc = bacc.Bacc("TRN2", debug=False)
    x = nc.dram_tensor("x", [B_LOC, C, H, W], dt.float32, kind="ExternalInput")
    nb = nc.dram_tensor("neighbors", [N_LOC, C, 9], dt.float32, kind="ExternalInput")
    out = nc.dram_tensor(
        "out", [B_LOC, N_LOC * C, H, W], dt.float32, kind="ExternalOutput"
    )

    # shift k = (r+1)*3 + (c+1) with r, c in {-1, 0, 1}
    RC = [(r, c) for r in (-1, 0, 1) for c in (-1, 0, 1)]

    def wstart(b, k):
        r, c = RC[k]
        return (2 + 33 * b - r) * 32 - c

    # bad (wrapped) column of a c-shifted plane: c=+1 -> col 0, c=-1 -> col 31
    def badcol(k):
        c = RC[k][1]
        return None if c == 0 else (0 if c == 1 else 31)

    with tile.TileContext(nc) as tc:
        with (
            tc.tile_pool(name="const", bufs=1) as cpool,
            tc.tile_pool(name="pp", bufs=2) as ppool,
            tc.tile_pool(name="dp", bufs=2) as dpool,
            tc.tile_pool(name="fp", bufs=2) as fpool,
        ):
            # neighbors scalars first (tiny DMA; gates first ops)
            nbt = cpool.tile([128, NT * 9], dt.float32, tag="nbt")
            nb_src = nb.ap().rearrange("(t nn) c k -> (nn c) t k", nn=4)
            nc.sync.dma_start(nbt[:].rearrange("p (t k) -> p t k", t=NT), nb_src)
            nbneg = cpool.tile([128, NT * 9], dt.float32, tag="nbneg")
            nc.scalar.mul(nbneg[:], nbt[:], -1.0)
            nbabs = cpool.tile([128, NT * 9], dt.float32, tag="nbabs")
            nc.scalar.activation(nbabs[:], nbt[:], AF.Abs, scale=1.0)

            def nbcol(nt, k):
                return nbt[:, nt * 9 + k : nt * 9 + k + 1]

            def nbnegcol(nt, k):
                return nbneg[:, nt * 9 + k : nt * 9 + k + 1]

            def nbabscol(nt, k):
                return nbabs[:, nt * 9 + k : nt * 9 + k + 1]

            # padded x tile (bf16, SWDGE cast loads) and pad-row memsets
            xpad = cpool.tile([128, FREE], dt.bfloat16, tag="xpad")
            xr = xpad[:].rearrange("p (r w) -> p r w", r=ROWS)
            nc.gpsimd.memset(xr[:, 0:2, :], 0.0)        # guard + top pad
            for b in range(1, B_LOC):
                nc.gpsimd.memset(xr[:, 33 * b + 1 : 33 * b + 2, :], 0.0)
            nc.gpsimd.memset(xr[:, ROWS - 3 : ROWS, :], 0.0)

            # interior loads (contiguous 1024 per (b, c)), SWDGE cast
            for b in range(B_LOC):
                for nn in range(4):
                    nc.gpsimd.dma_start(
                        xr[nn * 32 : (nn + 1) * 32, 2 + 33 * b : 34 + 33 * b, :],
                        x.ap()[b],
                    )

            def win(b, k):
                s = wstart(b, k)
                return xpad[:, s : s + 1024]

            def win4(k):
                # all-batch window [p, 4, 1024] (stride 33*32)
                s = wstart(0, k)
                return xpad[:, s : s + 4 * 1056].rearrange(
                    "p (b u) -> p b u", u=1056
                )[:, :, :1024]

            accs = {}
            out_v = out.ap().rearrange("b (t p) h w -> t p b (h w)", t=NT)

            # max(|n_k0|, |n_k6|) per nt: fixup value for the PAIR(k0,k6) slab
            nbabs_v = nbabs[:].rearrange("p (t k) -> p t k", t=NT)
            nbm06 = cpool.tile([128, NT], dt.float32, tag="nbm06")
            nc.vector.tensor_tensor(
                nbm06[:], nbabs_v[:, :, 0], nbabs_v[:, :, 6], Alu.max
            )

            # nts 0,1 "heavy" (DVE takes k0,k6 too; ACT 4 planes),
            # nts 2,3 "light" (ACT 6 planes) -- keeps ACT ahead of the
            # DVE fold pipeline.
            HEAVY = (True, True, True, True)
            K_ACT_H = (2, 8, 3, 5, 4)
            K_ACT_L = (0, 2, 3, 5, 6, 8)

            ds_all = {}
            pslab2_all = {}

            def emit_act(nt):
                ks = K_ACT_H if HEAVY[nt] else K_ACT_L
                ds = []
                for ki, k in enumerate(ks):
                    d = dpool.tile(
                        [128, B_LOC * H * W], dt.bfloat16, tag=f"d{ki}",
                        name=f"d{ki}_{nt}",
                    )
                    nc.scalar.activation(
                        d[:].rearrange("p (b u) -> p b u", b=B_LOC),
                        win4(k), AF.Abs, bias=nbnegcol(nt, k), scale=1.0,
                    )
                    ds.append(d)
                ds_all[nt] = ds

            def emit_chains(nt):
                pslab = ppool.tile(
                    [128, B_LOC * H * W], dt.bfloat16, tag="pslab"
                )
                ps_v = pslab[:].rearrange("p (b u) -> p b u", b=B_LOC)
                for b in range(B_LOC):
                    nc.vector._custom_dve(
                        PAIR, out=ps_v[:, b], in0=win(b, 1), in1=win(b, 7),
                        s0=nbcol(nt, 1), s1=nbcol(nt, 7),
                    )
                if HEAVY[nt]:
                    p2 = ppool.tile(
                        [128, B_LOC * H * W], dt.bfloat16, tag="pslab2"
                    )
                    p2_v = p2[:].rearrange("p (b u) -> p b u", b=B_LOC)
                    for b in range(B_LOC):
                        nc.vector._custom_dve(
                            PAIR, out=p2_v[:, b], in0=win(b, 0), in1=win(b, 6),
                            s0=nbcol(nt, 0), s1=nbcol(nt, 6),
                        )
                    pslab2_all[nt] = p2
                accs[nt] = pslab

            def fixup(view_owner, col, scol):
                dv = view_owner[:].rearrange(
                    "p (b h w) -> p b h w", b=B_LOC, h=H
                )[:, :, :, col : col + 1]
                nc.vector.tensor_scalar(dv, dv, 0.0, scol, Alu.mult, Alu.add)

            def emit_folds(nt):
                ks = K_ACT_H if HEAVY[nt] else K_ACT_L
                ds = ds_all.pop(nt)
                for ki, k in enumerate(ks):
                    if badcol(k) is not None:
                        fixup(ds[ki], badcol(k), nbabscol(nt, k))
                sz = [128, B_LOC * H * W]
                if HEAVY[nt]:
                    p2 = pslab2_all.pop(nt)
                    fixup(p2, 31, nbm06[:, nt : nt + 1])
                    e0 = fpool.tile(sz, dt.bfloat16, tag="s0", name=f"e0_{nt}")
                    e1 = fpool.tile(sz, dt.bfloat16, tag="s1", name=f"e1_{nt}")
                    nc.vector.tensor_tensor(e0[:], ds[0][:], ds[1][:], Alu.max)
                    nc.vector.tensor_tensor(e1[:], ds[2][:], ds[3][:], Alu.max)
                    a2 = fpool.tile(sz, dt.bfloat16, tag="s2", name=f"a2_{nt}")
                    nc.vector.tensor_tensor(a2[:], ds[4][:], accs[nt][:], Alu.max)
                    f = fpool.tile(sz, dt.bfloat16, tag="s0", name=f"f_{nt}")
                    nc.vector.tensor_tensor(f[:], e0[:], e1[:], Alu.max)
                    a3 = fpool.tile(sz, dt.bfloat16, tag="s1", name=f"a3_{nt}")
                    nc.vector.tensor_tensor(a3[:], a2[:], p2[:], Alu.max)
                    aF = fpool.tile(sz, dt.bfloat16, tag="s2", name=f"aF_{nt}")
                    a3v = a3[:].rearrange("p (b s) -> p b s", b=B_LOC)
                    fv = f[:].rearrange("p (b s) -> p b s", b=B_LOC)
                    aFv = aF[:].rearrange("p (b s) -> p b s", b=B_LOC)
                    for b in range(B_LOC):
                        nc.vector.tensor_tensor(
                            aFv[:, b], a3v[:, b], fv[:, b], Alu.max
                        )
                        nc.gpsimd.dma_start(out_v[nt][:, b], aFv[:, b])
                    return
                else:
                    e0 = fpool.tile(sz, dt.bfloat16, tag="s0", name=f"e0_{nt}")
                    e1 = fpool.tile(sz, dt.bfloat16, tag="s1", name=f"e1_{nt}")
                    e2 = fpool.tile(sz, dt.bfloat16, tag="s2", name=f"e2_{nt}")
                    nc.vector.tensor_tensor(e0[:], ds[0][:], ds[1][:], Alu.max)
                    nc.vector.tensor_tensor(e1[:], ds[2][:], ds[3][:], Alu.max)
                    nc.vector.tensor_tensor(e2[:], ds[4][:], ds[5][:], Alu.max)
                    f = fpool.tile(sz, dt.bfloat16, tag="s0", name=f"f_{nt}")
                    nc.vector.tensor_tensor(f[:], e0[:], e1[:], Alu.max)
                    a2 = fpool.tile(sz, dt.bfloat16, tag="s1", name=f"a2_{nt}")
                    nc.vector.tensor_tensor(a2[:], accs[nt][:], e2[:], Alu.max)
                    aF = fpool.tile(sz, dt.bfloat16, tag="s2", name=f"aF_{nt}")
                    nc.vector.tensor_tensor(aF[:], a2[:], f[:], Alu.max)


            for nt in range(NT):
                emit_act(nt)
                emit_chains(nt)
                if nt >= 1:
                    emit_folds(nt - 1)
            emit_folds(NT - 1)

    nc.compile()
    return nc


def _get_module():
    if "nc" not in _module_cache:
        _module_cache["nc"] = _build_module()
    return _module_cache["nc"]


def _run(x, neighbors, trace=False):
    from concourse import bass_utils

    x = np.ascontiguousarray(x, dtype=np.float32)
    neighbors = np.ascontiguousarray(neighbors, dtype=np.float32)
    in_maps = []
    for core in range(NCORES):
        bg, ng = divmod(core, NG)
        in_maps.append(
            {
                "x": x[bg * B_LOC : (bg + 1) * B_LOC],
                "neighbors": neighbors[ng * N_LOC : (ng + 1) * N_LOC],
            }
        )
    res = bass_utils.run_bass_kernel_spmd(
        _get_module(), in_maps, core_ids=list(range(NCORES)), trace=trace
    )
    out = np.empty((B, NUM * C, H, W), dtype=np.float32)
    for core in range(NCORES):
        bg, ng = divmod(core, NG)
        out[bg * B_LOC : (bg + 1) * B_LOC, ng * N_LOC * C : (ng + 1) * N_LOC * C] = (
            res.results[core]["out"]
        )
    return out, res


def kernel(x, neighbors):
    out, _ = _run(x, neighbors, trace=False)
    return out


# revision 18
# speedup vs baseline: 1.5157x; 1.0002x over previous
"""ConvNearestNeightbor Trainium2 kernel (hybrid ACT + custom fused DVE ops).

out[b, n*C+c, i, j] = max_k |x[b,c,i-r_k,j-c_k] - neighbors[n,c,k]|
over the 9 zero-padded 3x3 shifts (r_k, c_k).

Sharding: 8 cores = 4 batch-groups x 2 num-groups.
Per core: B_loc=4 batches, N_loc=16 codebook entries.
Partition layout: (nn in 0..3, c in 0..31) -> 128 partitions; codebook
tile nt in 0..3 selects n = nt*4+nn.

x layout: NO column padding -- a [128, 135*32] f32 tile, batch b interior
contiguous at rows 2+33b .. 33+33b (32x32), zero pad rows between
batches (vertical zero-padding is exact).  Every 3x3-shift "window" is
then a 1D contiguous crop at offset (2+33b-r)*32 - c, so input DMA is
4KB-run cheap and custom-DVE ops take windows as both operands.
Horizontal shifts wrap one column per row into the neighbouring row;
that single known column per shifted plane is overwritten with the
correct zero-pad value |n_k| (tiny strided tensor_scalar) before
folding.

Engine split per nt (measured rates: ACT 1.2G elem/s; DVE custom 1 elem/
cyc fused produce+abs+fold, TT bf16 2 elem/cyc):
 - ACT: planes {0,2,3,5,6,8} as |x-n| (Abs+bias, f32 in, bf16 out)
 - DVE: PAIR(k1,k7) per batch + ACC(k4) at 4096, column fixups, and
   6 bf16 tensor_tensor max folds at 4096.
Output: bf16 accumulator -> SWDGE cast DMA -> f32.
"""

import numpy as np

B, C, H, W = 16, 32, 32, 32
NUM = 32
NCORES = 8
BG, NG = 4, 2          # batch groups x num groups
B_LOC = B // BG        # 4
N_LOC = NUM // NG      # 16
NT = N_LOC // 4        # 4 codebook tiles of 4 n each
ROWS = 136             # guard + 4x(pad+32) + bottom pads/guards
FREE = ROWS * 32

K_ACT = (0, 2, 3, 5, 6, 8)

_module_cache = {}


def _register_ops():
    """Register the two fused abs-diff-max DVE ops (idempotent)."""
    import concourse.dve_ops as dve_ops
    from concourse.dve_spec import Spec, Src0, Src1, C0, C1, maxx, lower
    from concourse.dve_uop import DveOpSpec
    from concourse.dve_table_gen import dve_ver_for

    names = ("ABSD_MAX_PAIR_AK", "ABSD_MAX_ACC_AK")
    if names[0] in dve_ops._SUB_OPCODE_FOR_NAME:
        by_name = {op.name: op for op in dve_ops.OPS}
        return by_name[names[0]], by_name[names[1]]

    ver = dve_ver_for("TRN2")

    def mk(name, body, ref):
        spec = Spec(body=body, reference=ref)
        row = max(dve_ops._SUB_OPCODE_FOR_NAME.values()) + 1
        assert row < 0x20
        dve_ops._SUB_OPCODE_FOR_NAME[name] = row
        uops = lower(spec, ver=ver)
        sha = DveOpSpec(name=name, opcode=row, uops=uops, rd1_en=True).sha(ver)
        op = dve_ops.DveOp(name, spec, subdim=False, uops_sha={ver: sha})
        dve_ops.OPS.append(op)
        dve_ops.CUSTOM_DVE_SPECS[name] = spec
        return op

    pair = mk(
        names[0],
        maxx(maxx(Src0 - C0, C0 - Src0), maxx(Src1 - C1, C1 - Src1)),
        lambda in0, in1, s0, s1, imm2: np.maximum(
            np.abs(in0.astype(np.float32) - s0),
            np.abs(in1.astype(np.float32).reshape(in0.shape) - s1),
        ),
    )
    acc = mk(
        names[1],
        maxx(maxx(Src0 - C0, C0 - Src0), Src1),
        lambda in0, in1, s0, s1, imm2: np.maximum(
            np.abs(in0.astype(np.float32) - s0),
            in1.astype(np.float32).reshape(in0.shape),
        ),
    )
    return pair, acc


def _build_module():
    import concourse.bacc as bacc
    import concourse.mybir as mybir
    import concourse.tile as tile

    PAIR, ACC = _register_ops()

    dt = mybir.dt
    Alu = mybir.AluOpType
    AF = mybir.ActivationFunctionType

    nc = bacc.Bacc("TRN2", debug=False)
    x = nc.dram_tensor("x", [B_LOC, C, H, W], dt.float32, kind="ExternalInput")
    nb = nc.dram_tensor("neighbors", [N_LOC, C, 9], dt.float32, kind="ExternalInput")
    out = nc.dram_tensor(
        "out", [B_LOC, N_LOC * C, H, W], dt.float32, kind="ExternalOutput"
    )

    # shift k = (r+1)*3 + (c+1) with r, c in {-1, 0, 1}
    RC = [(r, c) for r in (-1, 0, 1) for c in (-1, 0, 1)]

    def wstart(b, k):
        r, c = RC[k]
        return (2 + 33 * b - r) * 32 - c

    # bad (wrapped) column of a c-shifted plane: c=+1 -> col 0, c=-1 -> col 31
    def badcol(k):
        c = RC[k][1]
        return None if c == 0 else (0 if c == 1 else 31)

    with tile.TileContext(nc) as tc:
        with (
            tc.tile_pool(name="const", bufs=1) as cpool,
            tc.tile_pool(name="pp", bufs=2) as ppool,
            tc.tile_pool(name="dp", bufs=2) as dpool,
            tc.tile_pool(name="fp", bufs=2) as fpool,
        ):
            # neighbors scalars first (tiny DMA; gates first ops)
            nbt = cpool.tile([128, NT * 9], dt.float32, tag="nbt")
            nb_src = nb.ap().rearrange("(t nn) c k -> (nn c) t k", nn=4)
            nc.sync.dma_start(nbt[:].rearrange("p (t k) -> p t k", t=NT), nb_src)
            nbneg = cpool.tile([128, NT * 9], dt.float32, tag="nbneg")
            nc.scalar.mul(nbneg[:], nbt[:], -1.0)
            nbabs = cpool.tile([128, NT * 9], dt.float32, tag="nbabs")
            nc.scalar.activation(nbabs[:], nbt[:], AF.Abs, scale=1.0)

            def nbcol(nt, k):
                return nbt[:, nt * 9 + k : nt * 9 + k + 1]

            def nbnegcol(nt, k):
                return nbneg[:, nt * 9 + k : nt * 9 + k + 1]

            def nbabscol(nt, k):
                return nbabs[:, nt * 9 + k : nt * 9 + k + 1]

            # padded x tile (bf16, SWDGE cast loads) and pad-row memsets
            xpad = cpool.tile([128, FREE], dt.bfloat16, tag="xpad")
            xr = xpad[:].rearrange("p (r w) -> p r w", r=ROWS)
            nc.vector.memset(xr[:, 0:2, :], 0.0)        # guard + top pad
            for b in range(1, B_LOC):
                nc.vector.memset(xr[:, 33 * b + 1 : 33 * b + 2, :], 0.0)
            nc.vector.memset(xr[:, ROWS - 3 : ROWS, :], 0.0)

            # interior loads (contiguous 1024 per (b, c)), SWDGE cast
            for b in range(B_LOC):
                for nn in range(4):
                    nc.gpsimd.dma_start(
                        xr[nn * 32 : (nn + 1) * 32, 2 + 33 * b : 34 + 33 * b, :],
                        x.ap()[b],
                    )

            def win(b, k):
                s = wstart(b, k)
                return xpad[:, s : s + 1024]

            def win4(k):
                # all-batch window [p, 4, 1024] (stride 33*32)
                s = wstart(0, k)
                return xpad[:, s : s + 4 * 1056].rearrange(
                    "p (b u) -> p b u", u=1056
                )[:, :, :1024]

            accs = {}
            out_v = out.ap().rearrange("b (t p) h w -> t p b (h w)", t=NT)

            # max(|n_k0|, |n_k6|) per nt: fixup value for the PAIR(k0,k6) slab
            nbabs_v = nbabs[:].rearrange("p (t k) -> p t k", t=NT)
            nbm06 = cpool.tile([128, NT], dt.float32, tag="nbm06")
            nc.vector.tensor_tensor(
                nbm06[:], nbabs_v[:, :, 0], nbabs_v[:, :, 6], Alu.max
            )

            # nts 0,1 "heavy" (DVE takes k0,k6 too; ACT 4 planes),
            # nts 2,3 "light" (ACT 6 planes) -- keeps ACT ahead of the
            # DVE fold pipeline.
            HEAVY = (True, True, True, True)
            K_ACT_H = (2, 8, 3, 5, 4)
            K_ACT_L = (0, 2, 3, 5, 6, 8)

            ds_all = {}
            pslab2_all = {}

            def emit_act(nt):
                ks = K_ACT_H if HEAVY[nt] else K_ACT_L
                ds = []
                for ki, k in enumerate(ks):
                    d = dpool.tile(
                        [128, B_LOC * H * W], dt.bfloat16, tag=f"d{ki}",
                        name=f"d{ki}_{nt}",
                    )
                    nc.scalar.activation(
                        d[:].rearrange("p (b u) -> p b u", b=B_LOC),
                        win4(k), AF.Abs, bias=nbnegcol(nt, k), scale=1.0,
                    )
                    ds.append(d)
                ds_all[nt] = ds

            def emit_chains(nt):
                pslab = ppool.tile(
                    [128, B_LOC * H * W], dt.bfloat16, tag="pslab"
                )
                ps_v = pslab[:].rearrange("p (b u) -> p b u", b=B_LOC)
                p2 = ppool.tile(
                    [128, B_LOC * H * W], dt.bfloat16, tag="pslab2"
                )
                p2_v = p2[:].rearrange("p (b u) -> p b u", b=B_LOC)
                for b in range(B_LOC):
                    nc.vector._custom_dve(
                        PAIR, out=ps_v[:, b], in0=win(b, 1), in1=win(b, 7),
                        s0=nbcol(nt, 1), s1=nbcol(nt, 7),
                    )
                    nc.vector._custom_dve(
                        PAIR, out=p2_v[:, b], in0=win(b, 0), in1=win(b, 6),
                        s0=nbcol(nt, 0), s1=nbcol(nt, 6),
                    )
                pslab2_all[nt] = p2
                accs[nt] = pslab

            def fixup(view_owner, col, scol):
                dv = view_owner[:].rearrange(
                    "p (b h w) -> p b h w", b=B_LOC, h=H
                )[:, :, :, col : col + 1]
                nc.vector.tensor_scalar(dv, dv, 0.0, scol, Alu.mult, Alu.add)

            def emit_folds(nt):
                ks = K_ACT_H if HEAVY[nt] else K_ACT_L
                ds = ds_all.pop(nt)
                for ki, k in enumerate(ks):
                    if badcol(k) is not None:
                        fixup(ds[ki], badcol(k), nbabscol(nt, k))
                sz = [128, B_LOC * H * W]
                if HEAVY[nt]:
                    p2 = pslab2_all.pop(nt)
                    fixup(p2, 31, nbm06[:, nt : nt + 1])
                    e0 = fpool.tile(sz, dt.bfloat16, tag="s0", name=f"e0_{nt}")
                    e1 = fpool.tile(sz, dt.bfloat16, tag="s1", name=f"e1_{nt}")
                    nc.vector.tensor_tensor(e0[:], ds[0][:], ds[1][:], Alu.max)
                    nc.vector.tensor_tensor(e1[:], ds[2][:], ds[3][:], Alu.max)
                    a2 = fpool.tile(sz, dt.bfloat16, tag="s2", name=f"a2_{nt}")
                    nc.vector.tensor_tensor(a2[:], ds[4][:], accs[nt][:], Alu.max)
                    f = fpool.tile(sz, dt.bfloat16, tag="s0", name=f"f_{nt}")
                    nc.vector.tensor_tensor(f[:], e0[:], e1[:], Alu.max)
                    a3 = fpool.tile(sz, dt.bfloat16, tag="s1", name=f"a3_{nt}")
                    nc.vector.tensor_tensor(a3[:], a2[:], p2[:], Alu.max)
                    aF = fpool.tile(sz, dt.bfloat16, tag="s2", name=f"aF_{nt}")
                    a3v = a3[:].rearrange("p (b s) -> p b s", b=B_LOC)
                    fv = f[:].rearrange("p (b s) -> p b s", b=B_LOC)
                    aFv = aF[:].rearrange("p (b s) -> p b s", b=B_LOC)
                    for b in range(B_LOC):
                        nc.vector.tensor_tensor(
                            aFv[:, b], a3v[:, b], fv[:, b], Alu.max
                        )
                        nc.gpsimd.dma_start(out_v[nt][:, b], aFv[:, b])
                    return
                else:
                    e0 = fpool.tile(sz, dt.bfloat16, tag="s0", name=f"e0_{nt}")
                    e1 = fpool.tile(sz, dt.bfloat16, tag="s1", name=f"e1_{nt}")
                    e2 = fpool.tile(sz, dt.bfloat16, tag="s2", name=f"e2_{nt}")
                    nc.vector.tensor_tensor(e0[:], ds[0][:], ds[1][:], Alu.max)
                    nc.vector.tensor_tensor(e1[:], ds[2][:], ds[3][:], Alu.max)
                    nc.vector.tensor_tensor(e2[:], ds[4][:], ds[5][:], Alu.max)
                    f = fpool.tile(sz, dt.bfloat16, tag="s0", name=f"f_{nt}")
                    nc.vector.tensor_tensor(f[:], e0[:], e1[:], Alu.max)
                    a2 = fpool.tile(sz, dt.bfloat16, tag="s1", name=f"a2_{nt}")
                    nc.vector.tensor_tensor(a2[:], accs[nt][:], e2[:], Alu.max)
                    aF = fpool.tile(sz, dt.bfloat16, tag="s2", name=f"aF_{nt}")
                    nc.vector.tensor_tensor(aF[:], a2[:], f[:], Alu.max)


            for nt in range(NT):
                emit_act(nt)
                emit_chains(nt)
                if nt >= 1:
                    emit_folds(nt - 1)
            emit_folds(NT - 1)

    nc.compile()
    return nc


def _get_module():
    if "nc" not in _module_cache:
        _module_cache["nc"] = _build_module()
    return _module_cache["nc"]


def _run(x, neighbors, trace=False):
    from concourse import bass_utils

    x = np.ascontiguousarray(x, dtype=np.float32)
    neighbors = np.ascontiguousarray(neighbors, dtype=np.float32)
    in_maps = []
    for core in range(NCORES):
        bg, ng = divmod(core, NG)
        in_maps.append(
            {
                "x": x[bg * B_LOC : (bg + 1) * B_LOC],
                "neighbors": neighbors[ng * N_LOC : (ng + 1) * N_LOC],
            }
        )
    res = bass_utils.run_bass_kernel_spmd(
        _get_module(), in_maps, core_ids=list(range(NCORES)), trace=trace
    )
    out = np.empty((B, NUM * C, H, W), dtype=np.float32)
    for core in range(NCORES):
        bg, ng = divmod(core, NG)
        out[bg * B_LOC : (bg + 1) * B_LOC, ng * N_LOC * C : (ng + 1) * N_LOC * C] = (
            res.results[core]["out"]
        )
    return out, res


def kernel(x, neighbors):
    out, _ = _run(x, neighbors, trace=False)
    return out


# revision 19
# speedup vs baseline: 1.5315x; 1.0104x over previous
"""ConvNearestNeightbor Trainium2 kernel (hybrid ACT + custom fused DVE ops).

out[b, n*C+c, i, j] = max_k |x[b,c,i-r_k,j-c_k] - neighbors[n,c,k]|
over the 9 zero-padded 3x3 shifts (r_k, c_k).

Sharding: 8 cores = 4 batch-groups x 2 num-groups.
Per core: B_loc=4 batches, N_loc=16 codebook entries.
Partition layout: (nn in 0..3, c in 0..31) -> 128 partitions; codebook
tile nt in 0..3 selects n = nt*4+nn.

x layout: NO column padding -- a [128, 135*32] f32 tile, batch b interior
contiguous at rows 2+33b .. 33+33b (32x32), zero pad rows between
batches (vertical zero-padding is exact).  Every 3x3-shift "window" is
then a 1D contiguous crop at offset (2+33b-r)*32 - c, so input DMA is
4KB-run cheap and custom-DVE ops take windows as both operands.
Horizontal shifts wrap one column per row into the neighbouring row;
that single known column per shifted plane is overwritten with the
correct zero-pad value |n_k| (tiny strided tensor_scalar) before
folding.

Engine split per nt (measured rates: ACT 1.2G elem/s; DVE custom 1 elem/
cyc fused produce+abs+fold, TT bf16 2 elem/cyc):
 - ACT: planes {0,2,3,5,6,8} as |x-n| (Abs+bias, f32 in, bf16 out)
 - DVE: PAIR(k1,k7) per batch + ACC(k4) at 4096, column fixups, and
   6 bf16 tensor_tensor max folds at 4096.
Output: bf16 accumulator -> SWDGE cast DMA -> f32.
"""

import numpy as np

B, C, H, W = 16, 32, 32, 32
NUM = 32
NCORES = 8
BG, NG = 4, 2          # batch groups x num groups
B_LOC = B // BG        # 4
N_LOC = NUM // NG      # 16
NT = N_LOC // 4        # 4 codebook tiles of 4 n each
ROWS = 136             # guard + 4x(pad+32) + bottom pads/guards
FREE = ROWS * 32

K_ACT = (0, 2, 3, 5, 6, 8)

_module_cache = {}


def _register_ops():
    """Register the two fused abs-diff-max DVE ops (idempotent)."""
    import concourse.dve_ops as dve_ops
    from concourse.dve_spec import Spec, Src0, Src1, C0, C1, maxx, lower
    from concourse.dve_uop import DveOpSpec
    from concourse.dve_table_gen import dve_ver_for

    names = ("ABSD_MAX_PAIR_AK", "ABSD_MAX_ACC_AK")
    if names[0] in dve_ops._SUB_OPCODE_FOR_NAME:
        by_name = {op.name: op for op in dve_ops.OPS}
        return by_name[names[0]], by_name[names[1]]

    ver = dve_ver_for("TRN2")

    def mk(name, body, ref):
        spec = Spec(body=body, reference=ref)
        row = max(dve_ops._SUB_OPCODE_FOR_NAME.values()) + 1
        assert row < 0x20
        dve_ops._SUB_OPCODE_FOR_NAME[name] = row
        uops = lower(spec, ver=ver)
        sha = DveOpSpec(name=name, opcode=row, uops=uops, rd1_en=True).sha(ver)
        op = dve_ops.DveOp(name, spec, subdim=False, uops_sha={ver: sha})
        dve_ops.OPS.append(op)
        dve_ops.CUSTOM_DVE_SPECS[name] = spec
        return op

    pair = mk(
        names[0],
        maxx(maxx(Src0 - C0, C0 - Src0), maxx(Src1 - C1, C1 - Src1)),
        lambda in0, in1, s0, s1, imm2: np.maximum(
            np.abs(in0.astype(np.float32) - s0),
            np.abs(in1.astype(np.float32).reshape(in0.shape) - s1),
        ),
    )
    acc = mk(
        names[1],
        maxx(maxx(Src0 - C0, C0 - Src0), Src1),
        lambda in0, in1, s0, s1, imm2: np.maximum(
            np.abs(in0.astype(np.float32) - s0),
            in1.astype(np.float32).reshape(in0.shape),
        ),
    )
    return pair, acc


def _build_module():
    import concourse.bacc as bacc
    import concourse.mybir as mybir
    import concourse.tile as tile

    PAIR, ACC = _register_ops()

    dt = mybir.dt
    Alu = mybir.AluOpType
    AF = mybir.ActivationFunctionType

    nc = bacc.Bacc("TRN2", debug=False)
    x = nc.dram_tensor("x", [B_LOC, C, H, W], dt.float32, kind="ExternalInput")
    nb = nc.dram_tensor("neighbors", [N_LOC, C, 9], dt.float32, kind="ExternalInput")
    out = nc.dram_tensor(
        "out", [B_LOC, N_LOC * C, H, W], dt.float32, kind="ExternalOutput"
    )

    # shift k = (r+1)*3 + (c+1) with r, c in {-1, 0, 1}
    RC = [(r, c) for r in (-1, 0, 1) for c in (-1, 0, 1)]

    def wstart(b, k):
        r, c = RC[k]
        return (2 + 33 * b - r) * 32 - c

    # bad (wrapped) column of a c-shifted plane: c=+1 -> col 0, c=-1 -> col 31
    def badcol(k):
        c = RC[k][1]
        return None if c == 0 else (0 if c == 1 else 31)

    with tile.TileContext(nc) as tc:
        with (
            tc.tile_pool(name="const", bufs=1) as cpool,
            tc.tile_pool(name="pp", bufs=2) as ppool,
            tc.tile_pool(name="dp", bufs=2) as dpool,
            tc.tile_pool(name="fp", bufs=2) as fpool,
        ):
            # neighbors scalars first (tiny DMA; gates first ops)
            nbt = cpool.tile([128, NT * 9], dt.float32, tag="nbt")
            nb_src = nb.ap().rearrange("(t nn) c k -> (nn c) t k", nn=4)
            nc.sync.dma_start(nbt[:].rearrange("p (t k) -> p t k", t=NT), nb_src)
            nbneg = cpool.tile([128, NT * 9], dt.float32, tag="nbneg")
            nc.scalar.mul(nbneg[:], nbt[:], -1.0)
            nbabs = cpool.tile([128, NT * 9], dt.float32, tag="nbabs")
            nc.scalar.activation(nbabs[:], nbt[:], AF.Abs, scale=1.0)

            def nbcol(nt, k):
                return nbt[:, nt * 9 + k : nt * 9 + k + 1]

            def nbnegcol(nt, k):
                return nbneg[:, nt * 9 + k : nt * 9 + k + 1]

            def nbabscol(nt, k):
                return nbabs[:, nt * 9 + k : nt * 9 + k + 1]

            # padded x tile (bf16, SWDGE cast loads) and pad-row memsets
            xpad = cpool.tile([128, FREE], dt.bfloat16, tag="xpad")
            xr = xpad[:].rearrange("p (r w) -> p r w", r=ROWS)
            nc.vector.memset(xr[:, 0:2, :], 0.0)        # guard + top pad
            for b in range(1, B_LOC):
                nc.vector.memset(xr[:, 33 * b + 1 : 33 * b + 2, :], 0.0)
            nc.vector.memset(xr[:, ROWS - 3 : ROWS, :], 0.0)

            # interior loads (contiguous 1024 per (b, c)), SWDGE cast
            for b in range(B_LOC):
                for nn in range(4):
                    nc.gpsimd.dma_start(
                        xr[nn * 32 : (nn + 1) * 32, 2 + 33 * b : 34 + 33 * b, :],
                        x.ap()[b],
                    )

            def win(b, k):
                s = wstart(b, k)
                return xpad[:, s : s + 1024]

            def win4(k):
                # all-batch window [p, 4, 1024] (stride 33*32)
                s = wstart(0, k)
                return xpad[:, s : s + 4 * 1056].rearrange(
                    "p (b u) -> p b u", u=1056
                )[:, :, :1024]

            accs = {}
            out_v = out.ap().rearrange("b (t p) h w -> t p b (h w)", t=NT)

            # max(|n_k0|, |n_k6|) per nt: fixup value for the PAIR(k0,k6) slab
            nbabs_v = nbabs[:].rearrange("p (t k) -> p t k", t=NT)
            nbm06 = cpool.tile([128, NT], dt.float32, tag="nbm06")
            nc.vector.tensor_tensor(
                nbm06[:], nbabs_v[:, :, 0], nbabs_v[:, :, 6], Alu.max
            )
            nbm258 = cpool.tile([128, NT], dt.float32, tag="nbm258")
            nc.vector.tensor_tensor(
                nbm258[:], nbabs_v[:, :, 2], nbabs_v[:, :, 8], Alu.max
            )
            nc.vector.tensor_tensor(
                nbm258[:], nbm258[:], nbabs_v[:, :, 5], Alu.max
            )

            # nts 0,1 "heavy" (DVE takes k0,k6 too; ACT 4 planes),
            # nts 2,3 "light" (ACT 6 planes) -- keeps ACT ahead of the
            # DVE fold pipeline.
            HEAVY = (True, True, True, True)
            K_ACT_H = (2, 8, 5, 3, 4)
            K_ACT_L = (0, 2, 3, 5, 6, 8)

            ds_all = {}
            pslab2_all = {}

            def emit_act(nt):
                ks = K_ACT_H if HEAVY[nt] else K_ACT_L
                ds = []
                for ki, k in enumerate(ks):
                    d = dpool.tile(
                        [128, B_LOC * H * W], dt.bfloat16, tag=f"d{ki}",
                        name=f"d{ki}_{nt}",
                    )
                    nc.scalar.activation(
                        d[:].rearrange("p (b u) -> p b u", b=B_LOC),
                        win4(k), AF.Abs, bias=nbnegcol(nt, k), scale=1.0,
                    )
                    ds.append(d)
                ds_all[nt] = ds

            def emit_chains(nt):
                pslab = ppool.tile(
                    [128, B_LOC * H * W], dt.bfloat16, tag="pslab"
                )
                ps_v = pslab[:].rearrange("p (b u) -> p b u", b=B_LOC)
                p2 = ppool.tile(
                    [128, B_LOC * H * W], dt.bfloat16, tag="pslab2"
                )
                p2_v = p2[:].rearrange("p (b u) -> p b u", b=B_LOC)
                for b in range(B_LOC):
                    nc.vector._custom_dve(
                        PAIR, out=ps_v[:, b], in0=win(b, 1), in1=win(b, 7),
                        s0=nbcol(nt, 1), s1=nbcol(nt, 7),
                    )
                    nc.vector._custom_dve(
                        PAIR, out=p2_v[:, b], in0=win(b, 0), in1=win(b, 6),
                        s0=nbcol(nt, 0), s1=nbcol(nt, 6),
                    )
                pslab2_all[nt] = p2
                accs[nt] = pslab

            def fixup(view_owner, col, scol):
                dv = view_owner[:].rearrange(
                    "p (b h w) -> p b h w", b=B_LOC, h=H
                )[:, :, :, col : col + 1]
                nc.vector.tensor_scalar(dv, dv, 0.0, scol, Alu.mult, Alu.add)

            def emit_folds(nt):
                ks = K_ACT_H if HEAVY[nt] else K_ACT_L
                ds = ds_all.pop(nt)
                sz = [128, B_LOC * H * W]
                p2 = pslab2_all.pop(nt)
                fixup(p2, 31, nbm06[:, nt : nt + 1])
                # ds = [d2, d8, d5, d3, d4]; 2/8/5 all wrap col 0
                e0 = fpool.tile(sz, dt.bfloat16, tag="s0", name=f"e0_{nt}")
                nc.vector.tensor_tensor(e0[:], ds[0][:], ds[1][:], Alu.max)
                e0b = fpool.tile(sz, dt.bfloat16, tag="s1", name=f"e0b_{nt}")
                nc.vector.tensor_tensor(e0b[:], e0[:], ds[2][:], Alu.max)
                fixup(e0b, 0, nbm258[:, nt : nt + 1])
                fixup(ds[3], 31, nbabscol(nt, 3))
                a2 = fpool.tile(sz, dt.bfloat16, tag="s2", name=f"a2_{nt}")
                nc.vector.tensor_tensor(a2[:], ds[4][:], accs[nt][:], Alu.max)
                f = fpool.tile(sz, dt.bfloat16, tag="s0", name=f"f_{nt}")
                nc.vector.tensor_tensor(f[:], e0b[:], ds[3][:], Alu.max)
                a3 = fpool.tile(sz, dt.bfloat16, tag="s1", name=f"a3_{nt}")
                nc.vector.tensor_tensor(a3[:], a2[:], p2[:], Alu.max)
                aF = fpool.tile(sz, dt.bfloat16, tag="s2", name=f"aF_{nt}")
                a3v = a3[:].rearrange("p (b s) -> p b s", b=B_LOC)
                fv = f[:].rearrange("p (b s) -> p b s", b=B_LOC)
                aFv = aF[:].rearrange("p (b s) -> p b s", b=B_LOC)
                for b in range(B_LOC):
                    nc.vector.tensor_tensor(
                        aFv[:, b], a3v[:, b], fv[:, b], Alu.max
                    )
                    nc.gpsimd.dma_start(out_v[nt][:, b], aFv[:, b])

            for nt in range(NT):
                emit_act(nt)
                emit_chains(nt)
                if nt >= 1:
                    emit_folds(nt - 1)
            emit_folds(NT - 1)

    nc.compile()
    return nc


def _get_module():
    if "nc" not in _module_cache:
        _module_cache["nc"] = _build_module()
    return _module_cache["nc"]


def _run(x, neighbors, trace=False):
    from concourse import bass_utils

    x = np.ascontiguousarray(x, dtype=np.float32)
    neighbors = np.ascontiguousarray(neighbors, dtype=np.float32)
    in_maps = []
    for core in range(NCORES):
        bg, ng = divmod(core, NG)
        in_maps.append(
            {
                "x": x[bg * B_LOC : (bg + 1) * B_LOC],
                "neighbors": neighbors[ng * N_LOC : (ng + 1) * N_LOC],
            }
        )
    res = bass_utils.run_bass_kernel_spmd(
        _get_module(), in_maps, core_ids=list(range(NCORES)), trace=trace
    )
    out = np.empty((B, NUM * C, H, W), dtype=np.float32)
    for core in range(NCORES):
        bg, ng = divmod(core, NG)
        out[bg * B_LOC : (bg + 1) * B_LOC, ng * N_LOC * C : (ng + 1) * N_LOC * C] = (
            res.results[core]["out"]
        )
    return out, res


def kernel(x, neighbors):
    out, _ = _run(x, neighbors, trace=False)
    return out
